# revision 4
# baseline (speedup 1.0000x reference)
"""Trainium2 Bass kernel for: out[b,h,w,i,k] = inputs[b,h,w,i] * u[i,k],
u[i,k] = beta[i,k]^2 / sum_k beta[i,k]^2.

Full inputs: inputs (4,256,256,32) f32, beta (32,8) f32.
Full output: (4,256,256,32,8) f32.

Data-parallel over the flattened 262144 spatial rows across 8 cores
(32768 rows/core); beta replicated. Per core: read 4MB, write 32MB.

Raw-bass (no Tile) pipeline:
  SP  : beta-bcast DMA, in-DMAs, out-DMAs (HWDGE)
  DVE : u = beta^2/rowsum(beta^2) preamble, then per-block broadcast-mul
Explicit semaphores, one per ring slot so each sem has at most one DMA
outstanding; all waits are standalone wait_ge instructions so no compute
instruction carries more than its single allowed sync command.
"""
import contextlib
import numpy as np

import concourse.bass as bass
import concourse.mybir as mybir
from concourse.bass_utils import run_bass_kernel_spmd

F32 = mybir.dt.float32
B, H, W, D, K = 4, 256, 256, 32, 8
F = D * K                     # 256
P = 128                       # SBUF partitions
N_CORES = 8
ROWS_TOTAL = B * H * W        # 262144
ROWS = ROWS_TOTAL // N_CORES  # 32768 per core


def _build(rows: int = ROWS, blk: int = 8, nbi: int = 4, nbo: int = 4,
           repeats: int = 1):
    rpi = blk * P
    assert rows % rpi == 0
    nt_data = rows // rpi
    nt = nt_data * repeats        # straight-line repeats for benchmarking
    fin = blk * D
    fout = blk * F

    nc = bass.Bass("TRN2", target_bir_lowering=False, debug=False)
    inp = nc.dram_tensor("inp", [rows, D], F32, kind="ExternalInput")
    beta = nc.dram_tensor("beta", [D, K], F32, kind="ExternalInput")
    out = nc.dram_tensor("out", [rows, F], F32, kind="ExternalOutput")

    inp_v0 = inp.ap().rearrange("(t j p) i -> t p j i", p=P, j=blk)
    out_v0 = out.ap().rearrange("(t j p) f -> t p j f", p=P, j=blk)
    inp_v = lambda t: inp_v0[t % nt_data]
    out_v = lambda t: out_v0[t % nt_data]

    with (
        nc.sbuf_tensor([P, nbi * fin], F32) as tin,
        nc.sbuf_tensor([P, nbo * fout], F32) as tout,
        nc.sbuf_tensor([P, 2 * fout + fin], F32) as scratch,
        nc.semaphore("beta_sem") as beta_sem,
        nc.semaphore("pre_sem") as pre_sem,
        nc.semaphore("dve_sem") as dve_sem,
        contextlib.ExitStack() as sem_stack,
        nc.Block() as block,
    ):
        isems = [sem_stack.enter_context(nc.semaphore(f"isem{i}")) for i in range(nbi)]
        osems = [sem_stack.enter_context(nc.semaphore(f"osem{i}")) for i in range(nbo)]
        u = scratch[:, 0:fout]
        bwork = scratch[:, fout:2 * fout]
        sums = scratch[:, 2 * fout:2 * fout + blk * D]

        def tin_s(t):
            return tin[:, (t % nbi) * fin:(t % nbi + 1) * fin]

        def tout_s(t):
            return tout[:, (t % nbo) * fout:(t % nbo + 1) * fout]

        @block.sync
        def _(sp):
            bsrc = beta.ap().rearrange("d k -> (d k)")
            bsrc = bsrc.unsqueeze(0).unsqueeze(0).broadcast_to([P, blk, F])
            sp.dma_start(
                out=bwork.rearrange("p (j f) -> p j f", j=blk), in_=bsrc
            ).then_inc(beta_sem, 16)
            for t in range(min(nbi, nt)):
                sp.dma_start(out=tin_s(t).rearrange("p (j i) -> p j i", j=blk),
                             in_=inp_v(t)).then_inc(isems[t % nbi], 16)
            for t in range(nt):
                sp.wait_ge(dve_sem, t + 1)
                sp.dma_start(out=out_v(t),
                             in_=tout_s(t).rearrange("p (j f) -> p j f", j=blk)
                             ).then_inc(osems[t % nbo], 16)
                if t + nbi < nt:
                    sp.dma_start(out=tin_s(t + nbi).rearrange("p (j i) -> p j i", j=blk),
                                 in_=inp_v(t + nbi)).then_inc(isems[(t + nbi) % nbi], 16)
            for s in range(min(nbo, nt)):
                uses = (nt - 1 - s) // nbo + 1
                sp.wait_ge(osems[s], 16 * uses)

        @block.vector
        def _(ve):
            ve.wait_ge(beta_sem, 16)
            bsq3 = bwork.rearrange("p (ji k) -> p ji k", k=K)
            ve.tensor_mul(bwork, bwork, bwork).then_inc(pre_sem, 1)
            ve.wait_ge(pre_sem, 1)
            ve.reduce_sum(sums, bsq3, axis=mybir.AxisListType.X).then_inc(pre_sem, 1)
            ve.wait_ge(pre_sem, 2)
            ve.reciprocal(sums, sums).then_inc(pre_sem, 1)
            ve.wait_ge(pre_sem, 3)
            u3 = u.rearrange("p (ji k) -> p ji k", k=K)
            ve.tensor_mul(u3, bsq3, sums.unsqueeze(-1).broadcast_to([P, blk * D, K])
                          ).then_inc(pre_sem, 1)
            ve.wait_ge(pre_sem, 4)
            for t in range(nt):
                ve.wait_ge(isems[t % nbi], 16 * (t // nbi + 1))
                if t >= nbo:
                    ve.wait_ge(osems[t % nbo], 16 * (t // nbo))
                ve.tensor_mul(
                    tout_s(t).rearrange("p (ji k) -> p ji k", k=K),
                    tin_s(t).unsqueeze(-1).broadcast_to([P, blk * D, K]),
                    u3,
                ).then_inc(dve_sem, 1)

    return nc


_NC_CACHE = {}


def _get_nc():
    if "nc" not in _NC_CACHE:
        _NC_CACHE["nc"] = _build()
    return _NC_CACHE["nc"]


def _run(inputs: np.ndarray, beta: np.ndarray, **spmd_kwargs):
    nc = _get_nc()
    flat = np.ascontiguousarray(inputs.reshape(ROWS_TOTAL, D))
    beta = np.ascontiguousarray(beta)
    in_maps = [
        {"inp": flat[c * ROWS:(c + 1) * ROWS], "beta": beta}
        for c in range(N_CORES)
    ]
    res = run_bass_kernel_spmd(nc, in_maps, list(range(N_CORES)), **spmd_kwargs)
    out = np.concatenate([res.results[c]["out"] for c in range(N_CORES)], axis=0)
    return out.reshape(B, H, W, D, K), res


def kernel(inputs: np.ndarray, beta: np.ndarray) -> np.ndarray:
    out, _ = _run(inputs, beta)
    return out


# revision 5
# speedup vs baseline: 1.0226x; 1.0226x over previous
"""Trainium2 Bass kernel for: out[b,h,w,i,k] = inputs[b,h,w,i] * u[i,k],
u[i,k] = beta[i,k]^2 / sum_k beta[i,k]^2.

Full inputs: inputs (4,256,256,32) f32, beta (32,8) f32.
Full output: (4,256,256,32,8) f32.

Data-parallel over the flattened 262144 spatial rows across 8 cores
(32768 rows/core); beta replicated. Per core: read 4MB, write 32MB.

Raw-bass (no Tile) pipeline:
  SP  : beta-bcast DMA, in-DMAs, out-DMAs (HWDGE)
  DVE : u = beta^2/rowsum(beta^2) preamble, then per-block broadcast-mul
Explicit semaphores, one per ring slot so each sem has at most one DMA
outstanding; all waits are standalone wait_ge instructions so no compute
instruction carries more than its single allowed sync command.
"""
import contextlib
import numpy as np

import concourse.bass as bass
import concourse.mybir as mybir
from concourse.bass_utils import run_bass_kernel_spmd

F32 = mybir.dt.float32
B, H, W, D, K = 4, 256, 256, 32, 8
F = D * K                     # 256
P = 128                       # SBUF partitions
N_CORES = 8
ROWS_TOTAL = B * H * W        # 262144
ROWS = ROWS_TOTAL // N_CORES  # 32768 per core


def _build(rows: int = ROWS, blk: int = 8, nbi: int = 4, nbo: int = 4,
           repeats: int = 1):
    rpi = blk * P
    assert rows % rpi == 0
    nt_data = rows // rpi
    nt = nt_data * repeats        # straight-line repeats for benchmarking
    fin = blk * D
    fout = blk * F

    nc = bass.Bass("TRN2", target_bir_lowering=False, debug=False)
    inp = nc.dram_tensor("inp", [rows, D], F32, kind="ExternalInput")
    beta = nc.dram_tensor("beta", [D, K], F32, kind="ExternalInput")
    out = nc.dram_tensor("out", [rows, F], F32, kind="ExternalOutput")

    inp_v0 = inp.ap().rearrange("(t j p) i -> t p j i", p=P, j=blk)
    out_v0 = out.ap().rearrange("(t j p) f -> t p j f", p=P, j=blk)
    inp_v = lambda t: inp_v0[t % nt_data]
    out_v = lambda t: out_v0[t % nt_data]

    with (
        nc.sbuf_tensor([P, nbi * fin], F32) as tin,
        nc.sbuf_tensor([P, nbo * fout], F32) as tout,
        nc.sbuf_tensor([P, 2 * fout + fin], F32) as scratch,
        nc.semaphore("beta_sem") as beta_sem,
        nc.semaphore("pre_sem") as pre_sem,
        nc.semaphore("dve_sem") as dve_sem,
        contextlib.ExitStack() as sem_stack,
        nc.Block() as block,
    ):
        nsem = 16  # rotate sems wider than the buffer rings to keep HW sem
        # counter values low (they appear to wrap/fault near 4096)
        isems = [sem_stack.enter_context(nc.semaphore(f"isem{i}")) for i in range(nsem)]
        osems = [sem_stack.enter_context(nc.semaphore(f"osem{i}")) for i in range(nsem)]
        u = scratch[:, 0:fout]
        bwork = scratch[:, fout:2 * fout]
        sums = scratch[:, 2 * fout:2 * fout + blk * D]

        def tin_s(t):
            return tin[:, (t % nbi) * fin:(t % nbi + 1) * fin]

        def tout_s(t):
            return tout[:, (t % nbo) * fout:(t % nbo + 1) * fout]

        @block.sync
        def _(sp):
            bsrc = beta.ap().rearrange("d k -> (d k)")
            bsrc = bsrc.unsqueeze(0).unsqueeze(0).broadcast_to([P, blk, F])
            sp.dma_start(
                out=bwork.rearrange("p (j f) -> p j f", j=blk), in_=bsrc
            ).then_inc(beta_sem, 16)
            for t in range(min(nbi, nt)):
                sp.dma_start(out=tin_s(t).rearrange("p (j i) -> p j i", j=blk),
                             in_=inp_v(t)).then_inc(isems[t % nsem], 16)
            for t in range(nt):
                sp.wait_ge(dve_sem, t + 1)
                sp.dma_start(out=out_v(t),
                             in_=tout_s(t).rearrange("p (j f) -> p j f", j=blk)
                             ).then_inc(osems[t % nsem], 16)
                if t + nbi < nt:
                    sp.dma_start(out=tin_s(t + nbi).rearrange("p (j i) -> p j i", j=blk),
                                 in_=inp_v(t + nbi)).then_inc(isems[(t + nbi) % nsem], 16)
            for s in range(min(nsem, nt)):
                uses = (nt - 1 - s) // nsem + 1
                sp.wait_ge(osems[s], 16 * uses)

        @block.vector
        def _(ve):
            ve.wait_ge(beta_sem, 16)
            bsq3 = bwork.rearrange("p (ji k) -> p ji k", k=K)
            ve.tensor_mul(bwork, bwork, bwork).then_inc(pre_sem, 1)
            ve.wait_ge(pre_sem, 1)
            ve.reduce_sum(sums, bsq3, axis=mybir.AxisListType.X).then_inc(pre_sem, 1)
            ve.wait_ge(pre_sem, 2)
            ve.reciprocal(sums, sums).then_inc(pre_sem, 1)
            ve.wait_ge(pre_sem, 3)
            u3 = u.rearrange("p (ji k) -> p ji k", k=K)
            ve.tensor_mul(u3, bsq3, sums.unsqueeze(-1).broadcast_to([P, blk * D, K])
                          ).then_inc(pre_sem, 1)
            ve.wait_ge(pre_sem, 4)
            for t in range(nt):
                ve.wait_ge(isems[t % nsem], 16 * (t // nsem + 1))
                if t >= nbo:
                    tp = t - nbo
                    ve.wait_ge(osems[tp % nsem], 16 * (tp // nsem + 1))
                ve.tensor_mul(
                    tout_s(t).rearrange("p (ji k) -> p ji k", k=K),
                    tin_s(t).unsqueeze(-1).broadcast_to([P, blk * D, K]),
                    u3,
                ).then_inc(dve_sem, 1)

    return nc


_NC_CACHE = {}


def _get_nc():
    if "nc" not in _NC_CACHE:
        _NC_CACHE["nc"] = _build()
    return _NC_CACHE["nc"]


def _run(inputs: np.ndarray, beta: np.ndarray, **spmd_kwargs):
    nc = _get_nc()
    flat = np.ascontiguousarray(inputs.reshape(ROWS_TOTAL, D))
    beta = np.ascontiguousarray(beta)
    in_maps = [
        {"inp": flat[c * ROWS:(c + 1) * ROWS], "beta": beta}
        for c in range(N_CORES)
    ]
    res = run_bass_kernel_spmd(nc, in_maps, list(range(N_CORES)), **spmd_kwargs)
    out = np.concatenate([res.results[c]["out"] for c in range(N_CORES)], axis=0)
    return out.reshape(B, H, W, D, K), res


def kernel(inputs: np.ndarray, beta: np.ndarray) -> np.ndarray:
    out, _ = _run(inputs, beta)
    return out


# revision 6
# speedup vs baseline: 1.0247x; 1.0021x over previous
"""Trainium2 Bass kernel for: out[b,h,w,i,k] = inputs[b,h,w,i] * u[i,k],
u[i,k] = beta[i,k]^2 / sum_k beta[i,k]^2.

Full inputs: inputs (4,256,256,32) f32, beta (32,8) f32.
Full output: (4,256,256,32,8) f32.

Data-parallel over the flattened 262144 spatial rows across 8 cores
(32768 rows/core); beta replicated. Per core: read 4MB, write 32MB.

Raw-bass (no Tile) pipeline:
  SP  : beta-bcast DMA, in-DMAs, out-DMAs (HWDGE)
  DVE : u = beta^2/rowsum(beta^2) preamble, then per-block broadcast-mul
Explicit semaphores, one per ring slot so each sem has at most one DMA
outstanding; all waits are standalone wait_ge instructions so no compute
instruction carries more than its single allowed sync command.
"""
import contextlib
import numpy as np

import concourse.bass as bass
import concourse.mybir as mybir
from concourse.bass_utils import run_bass_kernel_spmd

F32 = mybir.dt.float32
B, H, W, D, K = 4, 256, 256, 32, 8
F = D * K                     # 256
P = 128                       # SBUF partitions
N_CORES = 8
ROWS_TOTAL = B * H * W        # 262144
ROWS = ROWS_TOTAL // N_CORES  # 32768 per core


def _build(rows: int = ROWS, blk: int = 8, nbi: int = 4, nbo: int = 4,
           repeats: int = 1):
    rpi = blk * P
    assert rows % rpi == 0
    nt_data = rows // rpi
    nt = nt_data * repeats        # straight-line repeats for benchmarking
    fin = blk * D
    fout = blk * F

    nc = bass.Bass("TRN2", target_bir_lowering=False, debug=False)
    inp = nc.dram_tensor("inp", [rows, D], F32, kind="ExternalInput")
    beta = nc.dram_tensor("beta", [D, K], F32, kind="ExternalInput")
    out = nc.dram_tensor("out", [rows, F], F32, kind="ExternalOutput")

    inp_v0 = inp.ap().rearrange("(t j p) i -> t p j i", p=P, j=blk)
    out_v0 = out.ap().rearrange("(t j p) f -> t p j f", p=P, j=blk)
    inp_v = lambda t: inp_v0[t % nt_data]
    out_v = lambda t: out_v0[t % nt_data]

    with (
        nc.sbuf_tensor([P, nbi * fin], F32) as tin,
        nc.sbuf_tensor([P, nbo * fout], F32) as tout,
        nc.sbuf_tensor([P, 2 * fout + fin], F32) as scratch,
        nc.semaphore("beta_sem") as beta_sem,
        nc.semaphore("pre_sem") as pre_sem,
        nc.semaphore("dve_sem") as dve_sem,
        contextlib.ExitStack() as sem_stack,
        nc.Block() as block,
    ):
        nsem = 16  # rotate sems wider than the buffer rings to keep HW sem
        # counter values low (they appear to wrap/fault near 4096)
        isems = [sem_stack.enter_context(nc.semaphore(f"isem{i}")) for i in range(nsem)]
        osems = [sem_stack.enter_context(nc.semaphore(f"osem{i}")) for i in range(nsem)]
        u = scratch[:, 0:fout]
        bwork = scratch[:, fout:2 * fout]
        sums = scratch[:, 2 * fout:2 * fout + blk * D]

        def tin_s(t):
            return tin[:, (t % nbi) * fin:(t % nbi + 1) * fin]

        def tout_s(t):
            return tout[:, (t % nbo) * fout:(t % nbo + 1) * fout]

        @block.sync
        def _(sp):
            # out-DMAs only: keeps the big stores streaming on the SP HWDGE
            # ring while loads go down the ACT ring concurrently.
            for t in range(nt):
                sp.wait_ge(dve_sem, t + 1)
                sp.dma_start(out=out_v(t),
                             in_=tout_s(t).rearrange("p (j f) -> p j f", j=blk)
                             ).then_inc(osems[t % nsem], 16)
            for s in range(min(nsem, nt)):
                uses = (nt - 1 - s) // nsem + 1
                sp.wait_ge(osems[s], 16 * uses)

        @block.scalar
        def _(act):
            act.dma_start(
                out=bwork.rearrange("p (j f) -> p j f", j=blk),
                in_=beta.ap().rearrange("d k -> (d k)").unsqueeze(0).unsqueeze(0)
                    .broadcast_to([P, blk, F]),
            ).then_inc(beta_sem, 16)
            for t in range(nt):
                if t >= nbi:
                    act.wait_ge(dve_sem, t - nbi + 1)
                act.dma_start(out=tin_s(t).rearrange("p (j i) -> p j i", j=blk),
                              in_=inp_v(t)).then_inc(isems[t % nsem], 16)

        @block.vector
        def _(ve):
            ve.wait_ge(beta_sem, 16)
            bsq3 = bwork.rearrange("p (ji k) -> p ji k", k=K)
            ve.tensor_mul(bwork, bwork, bwork).then_inc(pre_sem, 1)
            ve.wait_ge(pre_sem, 1)
            ve.reduce_sum(sums, bsq3, axis=mybir.AxisListType.X).then_inc(pre_sem, 1)
            ve.wait_ge(pre_sem, 2)
            ve.reciprocal(sums, sums).then_inc(pre_sem, 1)
            ve.wait_ge(pre_sem, 3)
            u3 = u.rearrange("p (ji k) -> p ji k", k=K)
            ve.tensor_mul(u3, bsq3, sums.unsqueeze(-1).broadcast_to([P, blk * D, K])
                          ).then_inc(pre_sem, 1)
            ve.wait_ge(pre_sem, 4)
            for t in range(nt):
                ve.wait_ge(isems[t % nsem], 16 * (t // nsem + 1))
                if t >= nbo:
                    tp = t - nbo
                    ve.wait_ge(osems[tp % nsem], 16 * (tp // nsem + 1))
                ve.tensor_mul(
                    tout_s(t).rearrange("p (ji k) -> p ji k", k=K),
                    tin_s(t).unsqueeze(-1).broadcast_to([P, blk * D, K]),
                    u3,
                ).then_inc(dve_sem, 1)

    return nc


_NC_CACHE = {}


def _get_nc():
    if "nc" not in _NC_CACHE:
        _NC_CACHE["nc"] = _build()
    return _NC_CACHE["nc"]


def _run(inputs: np.ndarray, beta: np.ndarray, **spmd_kwargs):
    nc = _get_nc()
    flat = np.ascontiguousarray(inputs.reshape(ROWS_TOTAL, D))
    beta = np.ascontiguousarray(beta)
    in_maps = [
        {"inp": flat[c * ROWS:(c + 1) * ROWS], "beta": beta}
        for c in range(N_CORES)
    ]
    res = run_bass_kernel_spmd(nc, in_maps, list(range(N_CORES)), **spmd_kwargs)
    out = np.concatenate([res.results[c]["out"] for c in range(N_CORES)], axis=0)
    return out.reshape(B, H, W, D, K), res


def kernel(inputs: np.ndarray, beta: np.ndarray) -> np.ndarray:
    out, _ = _run(inputs, beta)
    return out


# revision 7
# speedup vs baseline: 1.2508x; 1.2207x over previous
"""Trainium2 Bass kernel for: out[b,h,w,i,k] = inputs[b,h,w,i] * u[i,k],
u[i,k] = beta[i,k]^2 / sum_k beta[i,k]^2.

Full inputs: inputs (4,256,256,32) f32, beta (32,8) f32.
Full output: (4,256,256,32,8) f32.

Data-parallel over the flattened 262144 spatial rows across 8 cores
(32768 rows/core); beta replicated. Per core: read 4MB, write 32MB.

Raw-bass (no Tile) pipeline:
  SP  : beta-bcast DMA, in-DMAs, out-DMAs (HWDGE)
  DVE : u = beta^2/rowsum(beta^2) preamble, then per-block broadcast-mul
Explicit semaphores, one per ring slot so each sem has at most one DMA
outstanding; all waits are standalone wait_ge instructions so no compute
instruction carries more than its single allowed sync command.
"""
import contextlib
import numpy as np

import concourse.bass as bass
import concourse.mybir as mybir
from concourse.bass_utils import run_bass_kernel_spmd

F32 = mybir.dt.float32
B, H, W, D, K = 4, 256, 256, 32, 8
F = D * K                     # 256
P = 128                       # SBUF partitions
N_CORES = 8
ROWS_TOTAL = B * H * W        # 262144
ROWS = ROWS_TOTAL // N_CORES  # 32768 per core


def _build(rows: int = ROWS, blk: int = 8, nbi: int = 4, nbo: int = 4,
           repeats: int = 1, bench_layout: int = 0):
    rpi = blk * P
    assert rows % rpi == 0
    nt_data = rows // rpi
    nt = nt_data * repeats        # straight-line repeats for benchmarking
    fin = blk * D
    fout = blk * F

    nc = bass.Bass("TRN2", target_bir_lowering=False, debug=False)
    inp = nc.dram_tensor("inp", [rows, D], F32, kind="ExternalInput")
    beta = nc.dram_tensor("beta", [D, K], F32, kind="ExternalInput")
    out = nc.dram_tensor("out", [rows, F], F32, kind="ExternalOutput")

    if bench_layout:
        # WRONG layout (bench only): maximally contiguous DMA APs to measure
        # the AP-efficiency headroom.
        inp_v0 = inp.ap().rearrange("(t p q) i -> t p (q i)", p=P, q=blk)
        out_v0 = out.ap().rearrange("(t p q) f -> t p (q f)", p=P, q=blk)
    else:
        inp_v0 = inp.ap().rearrange("(t j p) i -> t p j i", p=P, j=blk)
        out_v0 = out.ap().rearrange("(t j p) f -> t p j f", p=P, j=blk)
    inp_v = lambda t: inp_v0[t % nt_data]
    out_v = lambda t: out_v0[t % nt_data]

    with (
        nc.sbuf_tensor([P, nbi * fin], F32) as tin,
        nc.sbuf_tensor([P, nbo * fout], F32) as tout,
        nc.sbuf_tensor([P, 2 * fout + fin], F32) as scratch,
        nc.semaphore("beta_sem") as beta_sem,
        nc.semaphore("pre_sem") as pre_sem,
        nc.semaphore("dve_sem") as dve_sem,
        contextlib.ExitStack() as sem_stack,
        nc.Block() as block,
    ):
        nsem = 16  # rotate sems wider than the buffer rings to keep HW sem
        # counter values low (they appear to wrap/fault near 4096)
        isems = [sem_stack.enter_context(nc.semaphore(f"isem{i}")) for i in range(nsem)]
        osems = [sem_stack.enter_context(nc.semaphore(f"osem{i}")) for i in range(nsem)]
        u = scratch[:, 0:fout]
        bwork = scratch[:, fout:2 * fout]
        sums = scratch[:, 2 * fout:2 * fout + blk * D]

        def tin_s(t):
            return tin[:, (t % nbi) * fin:(t % nbi + 1) * fin]

        def tout_s(t):
            return tout[:, (t % nbo) * fout:(t % nbo + 1) * fout]

        @block.sync
        def _(sp):
            # out-DMAs only: keeps the big stores streaming on the SP HWDGE
            # ring while loads go down the ACT ring concurrently.
            for t in range(nt):
                sp.wait_ge(dve_sem, t + 1)
                src_ap = (tout_s(t) if bench_layout else
                          tout_s(t).rearrange("p (j f) -> p j f", j=blk))
                sp.dma_start(out=out_v(t), in_=src_ap
                             ).then_inc(osems[t % nsem], 16)
            for s in range(min(nsem, nt)):
                uses = (nt - 1 - s) // nsem + 1
                sp.wait_ge(osems[s], 16 * uses)

        @block.scalar
        def _(act):
            act.dma_start(
                out=bwork.rearrange("p (j f) -> p j f", j=blk),
                in_=beta.ap().rearrange("d k -> (d k)").unsqueeze(0).unsqueeze(0)
                    .broadcast_to([P, blk, F]),
            ).then_inc(beta_sem, 16)
            for t in range(nt):
                if t >= nbi:
                    act.wait_ge(dve_sem, t - nbi + 1)
                dst_ap = (tin_s(t) if bench_layout else
                          tin_s(t).rearrange("p (j i) -> p j i", j=blk))
                act.dma_start(out=dst_ap, in_=inp_v(t)).then_inc(isems[t % nsem], 16)

        @block.vector
        def _(ve):
            ve.wait_ge(beta_sem, 16)
            bsq3 = bwork.rearrange("p (ji k) -> p ji k", k=K)
            ve.tensor_mul(bwork, bwork, bwork).then_inc(pre_sem, 1)
            ve.wait_ge(pre_sem, 1)
            ve.reduce_sum(sums, bsq3, axis=mybir.AxisListType.X).then_inc(pre_sem, 1)
            ve.wait_ge(pre_sem, 2)
            ve.reciprocal(sums, sums).then_inc(pre_sem, 1)
            ve.wait_ge(pre_sem, 3)
            u3 = u.rearrange("p (ji k) -> p ji k", k=K)
            ve.tensor_mul(u3, bsq3, sums.unsqueeze(-1).broadcast_to([P, blk * D, K])
                          ).then_inc(pre_sem, 1)
            ve.wait_ge(pre_sem, 4)
            for t in range(nt):
                ve.wait_ge(isems[t % nsem], 16 * (t // nsem + 1))
                if t >= nbo:
                    tp = t - nbo
                    ve.wait_ge(osems[tp % nsem], 16 * (tp // nsem + 1))
                ve.tensor_mul(
                    tout_s(t).rearrange("p (ji k) -> p ji k", k=K),
                    tin_s(t).unsqueeze(-1).broadcast_to([P, blk * D, K]),
                    u3,
                ).then_inc(dve_sem, 1)

    return nc


_NC_CACHE = {}


def _get_nc():
    if "nc" not in _NC_CACHE:
        _NC_CACHE["nc"] = _build()
    return _NC_CACHE["nc"]


def _run(inputs: np.ndarray, beta: np.ndarray, **spmd_kwargs):
    nc = _get_nc()
    flat = np.ascontiguousarray(inputs.reshape(ROWS_TOTAL, D))
    beta = np.ascontiguousarray(beta)
    in_maps = [
        {"inp": flat[c * ROWS:(c + 1) * ROWS], "beta": beta}
        for c in range(N_CORES)
    ]
    res = run_bass_kernel_spmd(nc, in_maps, list(range(N_CORES)), **spmd_kwargs)
    out = np.concatenate([res.results[c]["out"] for c in range(N_CORES)], axis=0)
    return out.reshape(B, H, W, D, K), res


def kernel(inputs: np.ndarray, beta: np.ndarray) -> np.ndarray:
    out, _ = _run(inputs, beta)
    return out
